# revision 20
# baseline (speedup 1.0000x reference)
"""CascadedBranch (retrieval_knn) Trainium2 kernel.

Reference computation (B=256, K=8, Da=768, Dt=512, V=49408):
    proj = audio_kw @ W_proj + b_proj          # [B,K,Dt]
    bn   = batchnorm over (B,K) with gamma/beta
    cos  = normalize(bn) @ normalize(emb).T    # [B,K,V]
    prob = softmax(cos / 0.1)
    out  = prob @ emb                          # [B,K,Dt]

Strategy: shard the vocab axis V across the 8 cores (6400 rows each after
padding 49408 -> 51200). Each core computes, for all 2048 rows:
    projT (W.T @ audio.T, PSUM f32), batchnorm stats via bn_stats/bn_aggr,
    normalized bnT (in [d, row] layout so BN params are per-partition),
    scores sT[v,row] = embT_q8 @ bnT_n (fp8 stationary x bf16 moving),
    expt = exp(scores * 10/64) in fp8e4,
    u = sum_v expt*emb_q8 via fp8 DoubleRow matmuls (2 MACs/PE/cycle),
    s = sum_v expt via DVE accumulate + a 1-column PE matmul reduce.
No max-subtraction is needed: |cos|<=~1 so logits are in [-10.2, 10.2].
Host combines: out = (sum_c u_c / ESCALE) / (sum_c s_c - NPAD)  (the pad
rows contribute exactly exp(0)=1 to s and 0 to u).
b_proj is ignored: a linear bias cancels exactly inside batchnorm.

Row-norm trick: gamma/beta are host-prescaled by 1/sqrt(sum(g^2+b^2)) so
every bn row norm^2 lands near 1.0; the device then computes
rsqrt(norm^2) as two seed-1 Newton steps on the DVE -- no scalar-engine
Sqrt in the steady loop, so the scalar activation table never leaves Exp
and the in-order scalar queue can't head-of-line block on the norm chain.

Schedule: the vocab loop is software-pipelined one pair ahead (including
across row-chunk boundaries, alternating score-psum pools on a global
pair counter) so the scalar exp always hides under the next pair's score
stream. The next row chunk's norm chain runs on DVE/gpsimd during the
current chunk's vocab loop (reduce at pair 0, rsqrt+muls at pair 4).

Everything needed is hardcoded; no sibling imports.
"""

import numpy as np
import ml_dtypes

import concourse.bass as bass
import concourse.bacc as bacc
import concourse.tile as tile
from concourse import mybir
from concourse import bass_isa
from concourse.bass_utils import run_bass_kernel_spmd

F32 = mybir.dt.float32
BF16 = mybir.dt.bfloat16
F8 = mybir.dt.float8e4

B, K, DA, D, V = 256, 8, 768, 512, 49408
R = B * K              # 2048 rows
NCORES = 8
VS = 6400              # per-core vocab shard (padded)
NJ = VS // 256         # 25 vocab pairs of 2x128
NRC = 4                # row chunks of 512
RC = 512
NDC = D // 128         # 4 d-chunks
NKC = DA // 128        # 6 k-chunks
NPAD = VS * NCORES - V  # 1792 zero pad rows (all in core 7's shard)
VQ_TEMP = 0.1
BN_EPS = 1e-5
ESCALE = 256.0         # emb pre-scale for fp8 quantization
NSCALE = 64.0          # emb_n pre-scale for fp8 quantization


def _split_sync_waits(nc):
    """The walrus in this image rejects >1 sem-wait per instruction
    ("Too many sync wait commands"). Legalize by inserting single-wait
    Drain carriers immediately before any multi-wait instruction (same
    engine, same basic block position => identical synchronization)."""
    import orjson
    js = orjson.loads(mybir.module_to_json_bytes(nc.m))
    ctr = 0
    for func in js["functions"]:
        for bb in func["blocks"]:
            out = []
            changed = False
            for inst in bb["instructions"]:
                si = inst.get("sync_info")
                waits = (si or {}).get("on_wait") or []
                if len(waits) > 1:
                    changed = True
                    for w in waits[:-1]:
                        ctr += 1
                        carrier = {
                            "name": f"I-lsw-{ctr}",
                            "opcode": "Drain",
                            "engine": inst["engine"],
                            "ins": [],
                            "outs": [],
                            "sync_info": {"on_wait": [w], "on_update": []},
                        }
                        if "debug" in inst:
                            carrier["debug"] = inst["debug"]
                        out.append(carrier)
                    si["on_wait"] = [waits[-1]]
                out.append(inst)
            if changed:
                bb["instructions"] = out
    nc.m = mybir.module_from_json_bytes(orjson.dumps(js))
    return nc


def _patch_upload_artifacts():
    import concourse.bass_utils as bu
    bu.upload_artifacts = lambda tmpdir: "local://" + str(tmpdir)


def _build_kernel():
    nc = bacc.Bacc("TRN2", target_bir_lowering=False)

    # inputs, host-prepped into [128, ...] partition-major layouts
    audio_d = nc.dram_tensor("audioTb", [128, NRC, NKC, RC], BF16,
                             kind="ExternalInput")
    w_d = nc.dram_tensor("wb", [128, NKC, D], BF16, kind="ExternalInput")
    gamma_d = nc.dram_tensor("gammab", [128, NDC], F32, kind="ExternalInput")
    seed_d = nc.dram_tensor("rstdseed", [128, NDC], F32, kind="ExternalInput")
    beta_d = nc.dram_tensor("betab", [128, NDC], F32, kind="ExternalInput")
    # embTnb[p, j, b, dc, q] = NSCALE*normalize(emb)[vshard, d] (fp8e4)
    embtn_d = nc.dram_tensor("embTnb", [128, NJ, 2, NDC, 128], F8,
                             kind="ExternalInput")
    # embb[p, j, b, d] = ESCALE*emb[shard + j*256+b*128+p, d] in fp8e4
    emb_d = nc.dram_tensor("embb", [128, NJ, 2, D], F8, kind="ExternalInput")
    u_d = nc.dram_tensor("u", [R, D], F32, kind="ExternalOutput")
    s_d = nc.dram_tensor("s", [NRC, RC], F32, kind="ExternalOutput")

    with tile.TileContext(nc) as tc:
        with (
            tc.tile_pool(name="consts", bufs=1) as consts,
            tc.tile_pool(name="persist", bufs=1) as persist,
            tc.tile_pool(name="sqp", bufs=4) as sqp,
            tc.tile_pool(name="nrp", bufs=2) as nrp,
            tc.tile_pool(name="expp", bufs=12) as expp,
            tc.tile_pool(name="accp", bufs=2) as accp,
            tc.tile_pool(name="outp", bufs=4) as outp,
            tc.tile_pool(name="psA0", bufs=1, space="PSUM") as psA0,
            tc.tile_pool(name="psA1", bufs=1, space="PSUM") as psA1,
            tc.tile_pool(name="psB", bufs=4, space="PSUM") as psB,
        ):
            psA = (psA0, psA1)
            # ---- load inputs: few big DMAs for max HBM rate ----
            w_sb = consts.tile([128, NKC, D], BF16, tag="w")
            nc.sync.dma_start(out=w_sb[:, :, :], in_=w_d[:, :, :])
            audio_sb = consts.tile([128, NRC, NKC, RC], BF16, tag="audio")
            for rc in range(NRC):
                nc.sync.dma_start(out=audio_sb[:, rc, :, :],
                                  in_=audio_d[:, rc, :, :])
            gamma_sb = consts.tile([128, NDC], F32, tag="gamma")
            nc.sync.dma_start(out=gamma_sb[:, :], in_=gamma_d[:, :])
            beta_sb = consts.tile([128, NDC], F32, tag="beta")
            nc.sync.dma_start(out=beta_sb[:, :], in_=beta_d[:, :])
            seed_sb = consts.tile([128, NDC], F32, tag="seed")
            nc.sync.dma_start(out=seed_sb[:, :], in_=seed_d[:, :])
            embtn_sb = consts.tile([128, NJ, 2, NDC, 128], F8, tag="embtn")
            emb_sb = consts.tile([128, NJ, 2, D], F8, tag="emb")

            def emit_table_dma(ch):
                # issued from the scalar engine mid-phase-B, so the audio
                # and weight DMAs get the full HBM bandwidth first
                jj = slice(ch * 12, 25 if ch else 12)
                nc.scalar.dma_start(out=embtn_sb[:, jj, :, :, :],
                                    in_=embtn_d[:, jj, :, :, :])
                nc.scalar.dma_start(out=emb_sb[:, jj, :, :], in_=emb_d[:, jj, :, :])

            ones_bf = consts.tile([128, 1], BF16, tag="ones_bf")
            nc.vector.memset(ones_bf, 1.0)
            ones_row = consts.tile([1, 128], BF16, tag="ones_row")
            nc.vector.memset(ones_row, 1.0)

            projT = [persist.tile([128, R], BF16, tag=f"projT{dc}",
                                  name=f"projT{dc}") for dc in range(NDC)]
            stats = [persist.tile([128, NRC, 6], F32, tag=f"stats{dc}",
                                  name=f"stats{dc}") for dc in range(NDC)]
            bnnT = [persist.tile([128, R], BF16, tag=f"bnnT{dc}",
                                 name=f"bnnT{dc}") for dc in range(NDC)]
            sdc, bdc = [None] * NDC, [None] * NDC
            sq0 = [None] * NDC
            rs0 = slice(0, RC)

            # ---- phase B: projT = W.T @ audio.T (rc-major so each audio
            # DMA chunk feeds two back-to-back groups). bn_stats on DVE,
            # psum->SBUF copy on the scalar engine.
            for rc in range(NRC):
                rs = slice(rc * RC, (rc + 1) * RC)
                for dch in range(2):
                    g = rc * 2 + dch
                    ps = psA[g % 2].tile([128, 2, RC], F32, tag="ps")
                    for b in range(2):
                        dc = dch * 2 + b
                        for a in range(NKC):
                            nc.tensor.matmul(
                                ps[:, b, :],
                                w_sb[:, a, dc * 128:(dc + 1) * 128],
                                audio_sb[:, rc, a, :],
                                start=(a == 0),
                                stop=(a == NKC - 1),
                            )
                    for b in range(2):
                        dc = dch * 2 + b
                        nc.vector.bn_stats(out=stats[dc][:, rc, :], in_=ps[:, b, :])
                        nc.scalar.copy(projT[dc][:, rs], ps[:, b, :])
                    if g == 1 or g == 3:
                        emit_table_dma(g // 2)

            # ---- phase C: BN affine params, all d-chunks batched, DVE
            # only. rstd = rsqrt(var) via 2 Newton steps from the host
            # seed 1/sqrt(sum_k W_kd^2) (~3% off, exact after 2 steps).
            mv4 = persist.tile([128, NDC, 2], F32, tag="mv4")
            for dc in range(NDC):
                nc.vector.bn_aggr(out=mv4[:, dc, :], in_=stats[dc][:, :, :])
            var4 = mv4[:, :, 1]
            mean4 = mv4[:, :, 0]
            y4 = persist.tile([128, NDC], F32, tag="y4")
            t4 = persist.tile([128, NDC], F32, tag="t4")
            nc.vector.tensor_copy(y4[:, :], seed_sb[:, :])
            for _ in range(2):
                nc.vector.tensor_mul(t4[:, :], var4, y4[:, :])
                nc.vector.tensor_mul(t4[:, :], t4[:, :], y4[:, :])
                nc.vector.tensor_scalar(
                    out=t4[:, :], in0=t4[:, :], scalar1=-0.5, scalar2=1.5,
                    op0=mybir.AluOpType.mult, op1=mybir.AluOpType.add,
                )
                nc.vector.tensor_mul(y4[:, :], y4[:, :], t4[:, :])
            s_aff4 = persist.tile([128, NDC], F32, tag="saff4")
            nc.vector.tensor_mul(s_aff4[:, :], y4[:, :], gamma_sb[:, :])
            b_aff4 = persist.tile([128, NDC], F32, tag="baff4")
            nc.vector.tensor_mul(b_aff4[:, :], mean4, s_aff4[:, :])
            nc.vector.tensor_tensor(
                out=b_aff4[:, :], in0=beta_sb[:, :], in1=b_aff4[:, :],
                op=mybir.AluOpType.subtract,
            )
            for dc in range(NDC):
                sdc[dc] = s_aff4[:, dc:dc + 1]
                bdc[dc] = b_aff4[:, dc:dc + 1]

            # ---- rc0 norm tail: affine on DVE in parallel with
            # (s*proj+b)^2 on scalar; partition-reduce + broadcast on the
            # (idle) PE; rsqrt via seed-1 Newton on a [1,RC] strip.
            for dc in range(NDC):
                nc.vector.tensor_scalar(
                    out=bnnT[dc][:, rs0], in0=projT[dc][:, rs0],
                    scalar1=sdc[dc], scalar2=bdc[dc],
                    op0=mybir.AluOpType.mult, op1=mybir.AluOpType.add,
                )
                sqt = sqp.tile([128, RC], BF16, tag="sqt", name=f"sq0_{dc}")
                nc.scalar.activation(
                    out=sqt[:, :], in_=projT[dc][:, rs0],
                    func=mybir.ActivationFunctionType.Square,
                    bias=bdc[dc], scale=sdc[dc],
                )
                sq0[dc] = sqt
            n2ps = psA0.tile([128, 2, RC], F32, tag="ps", name="n2ps")
            for dc in range(NDC):
                nc.tensor.matmul(
                    n2ps[0:1, 0, :], ones_bf[:, 0:1], sq0[dc][:, :],
                    start=(dc == 0), stop=(dc == NDC - 1),
                )
            yr = nrp.tile([1, RC], F32, tag="yr")
            tr = nrp.tile([1, RC], F32, tag="tr")
            nc.vector.tensor_scalar(
                out=yr[:, :], in0=n2ps[0:1, 0, :], scalar1=-0.5, scalar2=1.5,
                op0=mybir.AluOpType.mult, op1=mybir.AluOpType.add,
            )
            nc.vector.tensor_mul(tr[:, :], n2ps[0:1, 0, :], yr[:, :])
            nc.vector.tensor_mul(tr[:, :], tr[:, :], yr[:, :])
            nc.vector.tensor_scalar(
                out=tr[:, :], in0=tr[:, :], scalar1=-0.5, scalar2=1.5,
                op0=mybir.AluOpType.mult, op1=mybir.AluOpType.add,
            )
            nc.vector.tensor_mul(yr[:, :], yr[:, :], tr[:, :])
            ybf = nrp.tile([1, RC], BF16, tag="ybf")
            nc.vector.tensor_copy(ybf[:, :], yr[:, :])
            rbcps = psA1.tile([128, 2, RC], F32, tag="ps", name="rbcps")
            nc.tensor.matmul(rbcps[:, 0, :], ones_row[:, :], ybf[:, :],
                             start=True, stop=True)
            for dc in range(NDC):
                nc.vector.tensor_mul(bnnT[dc][:, rs0], bnnT[dc][:, rs0],
                                     rbcps[:, 0, :])

            norm_state = {}

            def emit_norm_a(rc):
                # affine + squares + partition reduce (DVE + gpsimd only)
                rs = slice(rc * RC, (rc + 1) * RC)
                sqa = sqp.tile([128, RC], F32, tag="sqa", name=f"sqa{rc}")
                for dc in range(NDC):
                    nc.vector.tensor_scalar(
                        out=bnnT[dc][:, rs], in0=projT[dc][:, rs],
                        scalar1=sdc[dc], scalar2=bdc[dc],
                        op0=mybir.AluOpType.mult, op1=mybir.AluOpType.add,
                    )
                    sqt = sqp.tile([128, RC], F32, tag="sqf", name=f"sqf{rc}_{dc}")
                    nc.vector.tensor_mul(sqt[:, :], bnnT[dc][:, rs],
                                         bnnT[dc][:, rs])
                    if dc == 0:
                        sq_first = sqt
                    elif dc == 1:
                        nc.vector.tensor_add(sqa[:, :], sq_first[:, :], sqt[:, :])
                    else:
                        nc.vector.tensor_add(sqa[:, :], sqa[:, :], sqt[:, :])
                n2r = nrp.tile([128, RC], F32, tag="n2r", name=f"n2r{rc}")
                nc.gpsimd.partition_all_reduce(
                    n2r[:, :], sqa[:, :], channels=128,
                    reduce_op=bass_isa.ReduceOp.add,
                )
                norm_state[rc] = n2r

            def emit_norm_b(rc):
                # rsqrt(n2) via two seed-1 Newton steps (norms ~1 by the
                # host gamma/beta prescale), then scale bnnT. DVE only.
                rs = slice(rc * RC, (rc + 1) * RC)
                x = norm_state.pop(rc)
                y1 = nrp.tile([128, RC], F32, tag="rbc", name=f"y1_{rc}")
                nc.vector.tensor_scalar(
                    out=y1[:, :], in0=x[:, :], scalar1=-0.5, scalar2=1.5,
                    op0=mybir.AluOpType.mult, op1=mybir.AluOpType.add,
                )
                t = nrp.tile([128, RC], F32, tag="nt", name=f"nt{rc}")
                nc.vector.tensor_mul(t[:, :], x[:, :], y1[:, :])
                nc.vector.tensor_mul(t[:, :], t[:, :], y1[:, :])
                nc.vector.tensor_scalar(
                    out=t[:, :], in0=t[:, :], scalar1=-0.5, scalar2=1.5,
                    op0=mybir.AluOpType.mult, op1=mybir.AluOpType.add,
                )
                nc.vector.tensor_mul(y1[:, :], y1[:, :], t[:, :])
                for dc in range(NDC):
                    nc.vector.tensor_mul(bnnT[dc][:, rs], bnnT[dc][:, rs],
                                         y1[:, :])

            # ---- phase E: scores -> exp -> u, s ----
            NG = NRC * NJ

            def emit_scores(g):
                rc, j = divmod(g, NJ)
                rs = slice(rc * RC, (rc + 1) * RC)
                ps = psA[g % 2].tile([128, 2, RC], F32, tag="ps",
                                     name=f"sc{g}")
                for bidx in range(2):
                    for dc in range(NDC):
                        nc.tensor.matmul(
                            ps[:, bidx, :],
                            embtn_sb[:, j, bidx, dc, :],
                            bnnT[dc][:, rs],
                            start=(dc == 0), stop=(dc == NDC - 1),
                        )
                return ps

            acc2 = None
            psu = None
            pend = None
            ps_cur = emit_scores(0)
            for g in range(NG):
                rc, j = divmod(g, NJ)
                if j == 0:
                    if pend is not None:
                        # previous chunk's u copies, on DVE so the scalar
                        # exp stream is not delayed at the boundary
                        psu_old, rc_old = pend
                        for rsub in range(4):
                            ur = outp.tile([128, D], F32, tag="ur")
                            nc.vector.tensor_copy(ur[:, :], psu_old[rsub][:, :])
                            r0 = (rc_old * 4 + rsub) * 128
                            nc.sync.dma_start(out=u_d[r0:r0 + 128, :],
                                              in_=ur[:, :])
                        pend = None
                    acc2 = accp.tile([128, 2, RC], F32, tag="acc2",
                                     name=f"acc2_{rc}")
                    nc.vector.memset(acc2, 0.0)
                    if rc + 1 < NRC:
                        emit_norm_a(rc + 1)
                    psu = [psB.tile([128, D], F32, tag="psB",
                                    name=f"psu{rc}_{i}") for i in range(4)]
                if j == 4 and rc + 1 < NRC:
                    emit_norm_b(rc + 1)
                ps_nxt = emit_scores(g + 1) if g + 1 < NG else None
                expt2 = expp.tile([128, 2, RC], F8, tag="expt")
                for bidx in range(2):
                    nc.scalar.activation(
                        out=expt2[:, bidx, :], in_=ps_cur[:, bidx, :],
                        func=mybir.ActivationFunctionType.Exp,
                        scale=1.0 / (VQ_TEMP * NSCALE),
                    )
                for rsub in range(4):
                    nc.tensor.matmul(
                        psu[rsub][:, :],
                        expt2[:, :, rsub * 128:(rsub + 1) * 128],
                        emb_sb[:, j, :, :],
                        perf_mode=mybir.MatmulPerfMode.DoubleRow,
                        start=(j == 0), stop=(j == NJ - 1),
                    )
                nc.vector.tensor_add(acc2[:, :, :], acc2[:, :, :],
                                     expt2[:, :, :])
                ps_cur = ps_nxt
                if j == NJ - 1:
                    # epilogue for this row chunk. The final chunk copies u
                    # on the (now idle) scalar engine immediately; earlier
                    # chunks defer their copies to the next chunk's start.
                    if rc == NRC - 1:
                        for rsub in range(4):
                            ur = outp.tile([128, D], F32, tag="ur")
                            if rsub < 2:
                                nc.vector.tensor_copy(ur[:, :], psu[rsub][:, :])
                            else:
                                nc.scalar.copy(ur[:, :], psu[rsub][:, :])
                            r0 = (rc * 4 + rsub) * 128
                            nc.sync.dma_start(out=u_d[r0:r0 + 128, :],
                                              in_=ur[:, :])
                    else:
                        pend = (psu, rc)
                    sacc = sqp.tile([128, RC], BF16, tag="sacc",
                                    name=f"sacc{rc}")
                    nc.vector.tensor_add(sacc[:, :], acc2[:, 0, :],
                                         acc2[:, 1, :])
                    sps = psA[g % 2].tile([128, 2, RC], F32, tag="ps",
                                          name=f"sps{rc}")
                    nc.tensor.matmul(sps[0:1, 0, :], ones_bf[:, 0:1],
                                     sacc[:, :], start=True, stop=True)
                    srow = nrp.tile([1, RC], F32, tag="srow", name=f"srow{rc}")
                    nc.vector.tensor_copy(srow[:, :], sps[0:1, 0, :])
                    nc.sync.dma_start(out=s_d[rc:rc + 1, :], in_=srow[0:1, :])

    nc.compile()
    _split_sync_waits(nc)
    return nc


_NC = None


def kernel(audio_kw, W_proj, b_proj, bn_gamma, bn_beta, emb):
    global _NC
    audio_kw = np.asarray(audio_kw, dtype=np.float32)
    W_proj = np.asarray(W_proj, dtype=np.float32)
    bn_gamma = np.asarray(bn_gamma, dtype=np.float32)
    bn_beta = np.asarray(bn_beta, dtype=np.float32)
    emb = np.asarray(emb, dtype=np.float32)

    # host prep: partition-major device layouts
    audioT = np.ascontiguousarray(
        audio_kw.reshape(NRC, RC, NKC, 128).transpose(3, 0, 2, 1)
    ).astype(ml_dtypes.bfloat16)
    wb = np.ascontiguousarray(
        W_proj.reshape(NKC, 128, D).transpose(1, 0, 2)
    ).astype(ml_dtypes.bfloat16)
    # prescale gamma/beta so bn row norms^2 land near 1.0 (the device
    # computes rsqrt(norm^2) by seed-1 Newton; cos is scale-invariant)
    rho = 1.0 / np.sqrt(np.sum(bn_gamma ** 2 + bn_beta ** 2))
    gammab = np.ascontiguousarray((bn_gamma * rho).reshape(NDC, 128).T)
    betab = np.ascontiguousarray((bn_beta * rho).reshape(NDC, 128).T)
    # Newton seed for rstd: population variance of proj_d is ~|W_:,d|^2
    seedb = np.ascontiguousarray(
        (1.0 / np.sqrt(np.sum(W_proj ** 2, axis=0) + BN_EPS))
        .reshape(NDC, 128).T).astype(np.float32)

    norms = np.linalg.norm(emb, axis=1, keepdims=True)
    emb_n = emb / norms
    vtot = VS * NCORES
    embTn_pad = np.zeros((D, vtot), dtype=np.float32)
    embTn_pad[:, :V] = emb_n.T * NSCALE
    emb_pad = np.zeros((vtot, D), dtype=np.float32)
    emb_pad[:V] = emb * ESCALE

    in_maps = []
    for c in range(NCORES):
        # [dc, p, j, b, q] -> [p, j, b, dc, q]
        etn = np.ascontiguousarray(
            embTn_pad[:, c * VS:(c + 1) * VS]
            .reshape(NDC, 128, NJ, 2, 128).transpose(1, 2, 3, 0, 4)
        ).astype(ml_dtypes.float8_e4m3)
        # [j, b, p, d] -> [p, j, b, d]
        eb = np.ascontiguousarray(
            emb_pad[c * VS:(c + 1) * VS]
            .reshape(NJ, 2, 128, D).transpose(2, 0, 1, 3)
        ).astype(ml_dtypes.float8_e4m3)
        in_maps.append({
            "audioTb": audioT, "wb": wb, "gammab": gammab, "betab": betab,
            "rstdseed": seedb, "embTnb": etn, "embb": eb,
        })

    if _NC is None:
        _NC = _build_kernel()
    _patch_upload_artifacts()
    res = run_bass_kernel_spmd(_NC, in_maps, core_ids=list(range(NCORES)))

    u_tot = np.zeros((R, D), dtype=np.float64)
    s_tot = np.zeros((R,), dtype=np.float64)
    for c in range(NCORES):
        u_tot += res.results[c]["u"].astype(np.float64)
        s_tot += res.results[c]["s"].reshape(R).astype(np.float64)
    s_tot -= NPAD  # zero pad rows contribute exactly exp(0)=1 each
    out = (u_tot / ESCALE / s_tot[:, None]).astype(np.float32)
    return out.reshape(B, K, D)


# revision 21
# speedup vs baseline: 1.0419x; 1.0419x over previous
"""CascadedBranch (retrieval_knn) Trainium2 kernel.

Reference computation (B=256, K=8, Da=768, Dt=512, V=49408):
    proj = audio_kw @ W_proj + b_proj          # [B,K,Dt]
    bn   = batchnorm over (B,K) with gamma/beta
    cos  = normalize(bn) @ normalize(emb).T    # [B,K,V]
    prob = softmax(cos / 0.1)
    out  = prob @ emb                          # [B,K,Dt]

Strategy: shard the vocab axis V across the 8 cores (6400 rows each after
padding 49408 -> 51200). Each core computes, for all 2048 rows:
    projT (W.T @ audio.T, PSUM f32), batchnorm stats via bn_stats/bn_aggr,
    normalized bnT (in [d, row] layout so BN params are per-partition),
    scores sT[v,row] = embT_q8 @ bnT_n (fp8 stationary x bf16 moving),
    expt = exp(scores * 10/64) in fp8e4,
    u = sum_v expt*emb_q8 via fp8 DoubleRow matmuls (2 MACs/PE/cycle),
    s = sum_v expt via DVE accumulate + a 1-column PE matmul reduce.
No max-subtraction is needed: |cos|<=~1 so logits are in [-10.2, 10.2].
Host combines: out = (sum_c u_c / ESCALE) / (sum_c s_c - NPAD)  (the pad
rows contribute exactly exp(0)=1 to s and 0 to u).
b_proj is ignored: a linear bias cancels exactly inside batchnorm.

Row-norm trick: gamma/beta are host-prescaled by 1/sqrt(sum(g^2+b^2)) so
every bn row norm^2 lands near 1.0; the device then computes
rsqrt(norm^2) as two seed-1 Newton steps on the DVE -- no scalar-engine
Sqrt in the steady loop, so the scalar activation table never leaves Exp
and the in-order scalar queue can't head-of-line block on the norm chain.

Schedule: the vocab loop is software-pipelined one pair ahead (including
across row-chunk boundaries, alternating score-psum pools on a global
pair counter) so the scalar exp always hides under the next pair's score
stream. The next row chunk's norm chain runs on DVE/gpsimd during the
current chunk's vocab loop (reduce at pair 0, rsqrt+muls at pair 4).

Everything needed is hardcoded; no sibling imports.
"""

import numpy as np
import ml_dtypes

import concourse.bass as bass
import concourse.bacc as bacc
import concourse.tile as tile
from concourse import mybir
from concourse import bass_isa
from concourse.bass_utils import run_bass_kernel_spmd

F32 = mybir.dt.float32
BF16 = mybir.dt.bfloat16
F8 = mybir.dt.float8e4

B, K, DA, D, V = 256, 8, 768, 512, 49408
R = B * K              # 2048 rows
NCORES = 8
VS = 6400              # per-core vocab shard (padded)
NJ = VS // 256         # 25 vocab pairs of 2x128
NRC = 4                # row chunks of 512
RC = 512
NDC = D // 128         # 4 d-chunks
NKC = DA // 128        # 6 k-chunks
NPAD = VS * NCORES - V  # 1792 zero pad rows (all in core 7's shard)
VQ_TEMP = 0.1
BN_EPS = 1e-5
ESCALE = 256.0         # emb pre-scale for fp8 quantization
NSCALE = 64.0          # emb_n pre-scale for fp8 quantization


def _split_sync_waits(nc):
    """The walrus in this image rejects >1 sem-wait per instruction
    ("Too many sync wait commands"). Legalize by inserting single-wait
    Drain carriers immediately before any multi-wait instruction (same
    engine, same basic block position => identical synchronization)."""
    import orjson
    js = orjson.loads(mybir.module_to_json_bytes(nc.m))
    ctr = 0
    for func in js["functions"]:
        for bb in func["blocks"]:
            out = []
            changed = False
            for inst in bb["instructions"]:
                si = inst.get("sync_info")
                waits = (si or {}).get("on_wait") or []
                if len(waits) > 1:
                    changed = True
                    for w in waits[:-1]:
                        ctr += 1
                        carrier = {
                            "name": f"I-lsw-{ctr}",
                            "opcode": "Drain",
                            "engine": inst["engine"],
                            "ins": [],
                            "outs": [],
                            "sync_info": {"on_wait": [w], "on_update": []},
                        }
                        if "debug" in inst:
                            carrier["debug"] = inst["debug"]
                        out.append(carrier)
                    si["on_wait"] = [waits[-1]]
                out.append(inst)
            if changed:
                bb["instructions"] = out
    nc.m = mybir.module_from_json_bytes(orjson.dumps(js))
    return nc


def _patch_upload_artifacts():
    import concourse.bass_utils as bu
    bu.upload_artifacts = lambda tmpdir: "local://" + str(tmpdir)


def _build_kernel():
    nc = bacc.Bacc("TRN2", target_bir_lowering=False)

    # inputs, host-prepped into [128, ...] partition-major layouts
    audio_d = nc.dram_tensor("audioTb", [128, NRC, NKC, RC], BF16,
                             kind="ExternalInput")
    w_d = nc.dram_tensor("wb", [128, NKC, D], BF16, kind="ExternalInput")
    gamma_d = nc.dram_tensor("gammab", [128, NDC], F32, kind="ExternalInput")
    seed_d = nc.dram_tensor("rstdseed", [128, NDC], F32, kind="ExternalInput")
    beta_d = nc.dram_tensor("betab", [128, NDC], F32, kind="ExternalInput")
    # embTnb[p, j, b, dc, q] = NSCALE*normalize(emb)[vshard, d] (fp8e4)
    embtn_d = nc.dram_tensor("embTnb", [128, NJ, 2, NDC, 128], F8,
                             kind="ExternalInput")
    # embb[p, j, b, d] = ESCALE*emb[shard + j*256+b*128+p, d] in fp8e4
    emb_d = nc.dram_tensor("embb", [128, NJ, 2, D], F8, kind="ExternalInput")
    u_d = nc.dram_tensor("u", [R, D], F32, kind="ExternalOutput")
    s_d = nc.dram_tensor("s", [NRC, RC], F32, kind="ExternalOutput")

    with tile.TileContext(nc) as tc:
        with (
            tc.tile_pool(name="consts", bufs=1) as consts,
            tc.tile_pool(name="persist", bufs=1) as persist,
            tc.tile_pool(name="sqp", bufs=4) as sqp,
            tc.tile_pool(name="nrp", bufs=2) as nrp,
            tc.tile_pool(name="expp", bufs=12) as expp,
            tc.tile_pool(name="accp", bufs=2) as accp,
            tc.tile_pool(name="outp", bufs=4) as outp,
            tc.tile_pool(name="psA0", bufs=1, space="PSUM") as psA0,
            tc.tile_pool(name="psA1", bufs=1, space="PSUM") as psA1,
            tc.tile_pool(name="psB", bufs=4, space="PSUM") as psB,
        ):
            psA = (psA0, psA1)
            # ---- load inputs: few big DMAs for max HBM rate ----
            w_sb = consts.tile([128, NKC, D], BF16, tag="w")
            nc.sync.dma_start(out=w_sb[:, :, :], in_=w_d[:, :, :])
            audio_sb = consts.tile([128, NRC, NKC, RC], BF16, tag="audio")
            for rc in range(NRC):
                nc.sync.dma_start(out=audio_sb[:, rc, :, :],
                                  in_=audio_d[:, rc, :, :])
            gamma_sb = consts.tile([128, NDC], F32, tag="gamma")
            nc.sync.dma_start(out=gamma_sb[:, :], in_=gamma_d[:, :])
            beta_sb = consts.tile([128, NDC], F32, tag="beta")
            nc.sync.dma_start(out=beta_sb[:, :], in_=beta_d[:, :])
            seed_sb = consts.tile([128, NDC], F32, tag="seed")
            nc.sync.dma_start(out=seed_sb[:, :], in_=seed_d[:, :])
            embtn_sb = consts.tile([128, NJ, 2, NDC, 128], F8, tag="embtn")
            emb_sb = consts.tile([128, NJ, 2, D], F8, tag="emb")
            for ch in range(2):
                jj = slice(ch * 12, 25 if ch else 12)
                nc.sync.dma_start(out=embtn_sb[:, jj, :, :, :],
                                  in_=embtn_d[:, jj, :, :, :])
                nc.sync.dma_start(out=emb_sb[:, jj, :, :], in_=emb_d[:, jj, :, :])

            ones_bf = consts.tile([128, 1], BF16, tag="ones_bf")
            nc.vector.memset(ones_bf, 1.0)
            ones_row = consts.tile([1, 128], BF16, tag="ones_row")
            nc.vector.memset(ones_row, 1.0)

            projT = [persist.tile([128, R], BF16, tag=f"projT{dc}",
                                  name=f"projT{dc}") for dc in range(NDC)]
            stats = [persist.tile([128, NRC, 6], F32, tag=f"stats{dc}",
                                  name=f"stats{dc}") for dc in range(NDC)]
            bnnT = [persist.tile([128, R], BF16, tag=f"bnnT{dc}",
                                 name=f"bnnT{dc}") for dc in range(NDC)]
            sdc, bdc = [None] * NDC, [None] * NDC
            sq0 = [None] * NDC
            rs0 = slice(0, RC)

            # ---- phase B: projT = W.T @ audio.T (rc-major so each audio
            # DMA chunk feeds two back-to-back groups). bn_stats on DVE,
            # psum->SBUF copy on the scalar engine.
            for rc in range(NRC):
                rs = slice(rc * RC, (rc + 1) * RC)
                for dch in range(2):
                    g = rc * 2 + dch
                    ps = psA[g % 2].tile([128, 2, RC], F32, tag="ps")
                    for b in range(2):
                        dc = dch * 2 + b
                        for a in range(NKC):
                            nc.tensor.matmul(
                                ps[:, b, :],
                                w_sb[:, a, dc * 128:(dc + 1) * 128],
                                audio_sb[:, rc, a, :],
                                start=(a == 0),
                                stop=(a == NKC - 1),
                            )
                    for b in range(2):
                        dc = dch * 2 + b
                        nc.vector.bn_stats(out=stats[dc][:, rc, :], in_=ps[:, b, :])
                        nc.scalar.copy(projT[dc][:, rs], ps[:, b, :])

            # ---- phase C: BN affine params, all d-chunks batched, DVE
            # only. rstd = rsqrt(var) via 2 Newton steps from the host
            # seed 1/sqrt(sum_k W_kd^2) (~3% off, exact after 2 steps).
            mv4 = persist.tile([128, NDC, 2], F32, tag="mv4")
            for dc in range(NDC):
                nc.vector.bn_aggr(out=mv4[:, dc, :], in_=stats[dc][:, :, :])
            var4 = mv4[:, :, 1]
            mean4 = mv4[:, :, 0]
            y4 = persist.tile([128, NDC], F32, tag="y4")
            t4 = persist.tile([128, NDC], F32, tag="t4")
            nc.vector.tensor_copy(y4[:, :], seed_sb[:, :])
            for _ in range(2):
                nc.vector.tensor_mul(t4[:, :], var4, y4[:, :])
                nc.vector.tensor_mul(t4[:, :], t4[:, :], y4[:, :])
                nc.vector.tensor_scalar(
                    out=t4[:, :], in0=t4[:, :], scalar1=-0.5, scalar2=1.5,
                    op0=mybir.AluOpType.mult, op1=mybir.AluOpType.add,
                )
                nc.vector.tensor_mul(y4[:, :], y4[:, :], t4[:, :])
            s_aff4 = persist.tile([128, NDC], F32, tag="saff4")
            nc.vector.tensor_mul(s_aff4[:, :], y4[:, :], gamma_sb[:, :])
            b_aff4 = persist.tile([128, NDC], F32, tag="baff4")
            nc.vector.tensor_mul(b_aff4[:, :], mean4, s_aff4[:, :])
            nc.vector.tensor_tensor(
                out=b_aff4[:, :], in0=beta_sb[:, :], in1=b_aff4[:, :],
                op=mybir.AluOpType.subtract,
            )
            for dc in range(NDC):
                sdc[dc] = s_aff4[:, dc:dc + 1]
                bdc[dc] = b_aff4[:, dc:dc + 1]

            # ---- rc0 norm tail: affine on DVE in parallel with
            # (s*proj+b)^2 on scalar; partition-reduce + broadcast on the
            # (idle) PE; rsqrt via seed-1 Newton on a [1,RC] strip.
            for dc in range(NDC):
                nc.vector.tensor_scalar(
                    out=bnnT[dc][:, rs0], in0=projT[dc][:, rs0],
                    scalar1=sdc[dc], scalar2=bdc[dc],
                    op0=mybir.AluOpType.mult, op1=mybir.AluOpType.add,
                )
                sqt = sqp.tile([128, RC], BF16, tag="sqt", name=f"sq0_{dc}")
                nc.scalar.activation(
                    out=sqt[:, :], in_=projT[dc][:, rs0],
                    func=mybir.ActivationFunctionType.Square,
                    bias=bdc[dc], scale=sdc[dc],
                )
                sq0[dc] = sqt
            n2ps = psA0.tile([128, 2, RC], F32, tag="ps", name="n2ps")
            for dc in range(NDC):
                nc.tensor.matmul(
                    n2ps[0:1, 0, :], ones_bf[:, 0:1], sq0[dc][:, :],
                    start=(dc == 0), stop=(dc == NDC - 1),
                )
            yr = nrp.tile([1, RC], F32, tag="yr")
            tr = nrp.tile([1, RC], F32, tag="tr")
            nc.vector.tensor_scalar(
                out=yr[:, :], in0=n2ps[0:1, 0, :], scalar1=-0.5, scalar2=1.5,
                op0=mybir.AluOpType.mult, op1=mybir.AluOpType.add,
            )
            nc.vector.tensor_mul(tr[:, :], n2ps[0:1, 0, :], yr[:, :])
            nc.vector.tensor_mul(tr[:, :], tr[:, :], yr[:, :])
            nc.vector.tensor_scalar(
                out=tr[:, :], in0=tr[:, :], scalar1=-0.5, scalar2=1.5,
                op0=mybir.AluOpType.mult, op1=mybir.AluOpType.add,
            )
            nc.vector.tensor_mul(yr[:, :], yr[:, :], tr[:, :])
            ybf = nrp.tile([1, RC], BF16, tag="ybf")
            nc.vector.tensor_copy(ybf[:, :], yr[:, :])
            rbcps = psA1.tile([128, 2, RC], F32, tag="ps", name="rbcps")
            nc.tensor.matmul(rbcps[:, 0, :], ones_row[:, :], ybf[:, :],
                             start=True, stop=True)
            for dc in range(NDC):
                nc.vector.tensor_mul(bnnT[dc][:, rs0], bnnT[dc][:, rs0],
                                     rbcps[:, 0, :])

            norm_state = {}

            def emit_norm_a(rc):
                # affine + squares + partition reduce (DVE + gpsimd only)
                rs = slice(rc * RC, (rc + 1) * RC)
                sqa = sqp.tile([128, RC], F32, tag="sqa", name=f"sqa{rc}")
                for dc in range(NDC):
                    nc.vector.tensor_scalar(
                        out=bnnT[dc][:, rs], in0=projT[dc][:, rs],
                        scalar1=sdc[dc], scalar2=bdc[dc],
                        op0=mybir.AluOpType.mult, op1=mybir.AluOpType.add,
                    )
                    sqt = sqp.tile([128, RC], F32, tag="sqf", name=f"sqf{rc}_{dc}")
                    nc.vector.tensor_mul(sqt[:, :], bnnT[dc][:, rs],
                                         bnnT[dc][:, rs])
                    if dc == 0:
                        sq_first = sqt
                    elif dc == 1:
                        nc.vector.tensor_add(sqa[:, :], sq_first[:, :], sqt[:, :])
                    else:
                        nc.vector.tensor_add(sqa[:, :], sqa[:, :], sqt[:, :])
                n2r = nrp.tile([128, RC], F32, tag="n2r", name=f"n2r{rc}")
                nc.gpsimd.partition_all_reduce(
                    n2r[:, :], sqa[:, :], channels=128,
                    reduce_op=bass_isa.ReduceOp.add,
                )
                norm_state[rc] = n2r

            def emit_norm_b(rc):
                # rsqrt(n2) via two seed-1 Newton steps (norms ~1 by the
                # host gamma/beta prescale), then scale bnnT. DVE only.
                rs = slice(rc * RC, (rc + 1) * RC)
                x = norm_state.pop(rc)
                y1 = nrp.tile([128, RC], F32, tag="rbc", name=f"y1_{rc}")
                nc.vector.tensor_scalar(
                    out=y1[:, :], in0=x[:, :], scalar1=-0.5, scalar2=1.5,
                    op0=mybir.AluOpType.mult, op1=mybir.AluOpType.add,
                )
                t = nrp.tile([128, RC], F32, tag="nt", name=f"nt{rc}")
                nc.vector.tensor_mul(t[:, :], x[:, :], y1[:, :])
                nc.vector.tensor_mul(t[:, :], t[:, :], y1[:, :])
                nc.vector.tensor_scalar(
                    out=t[:, :], in0=t[:, :], scalar1=-0.5, scalar2=1.5,
                    op0=mybir.AluOpType.mult, op1=mybir.AluOpType.add,
                )
                nc.vector.tensor_mul(y1[:, :], y1[:, :], t[:, :])
                for dc in range(NDC):
                    nc.vector.tensor_mul(bnnT[dc][:, rs], bnnT[dc][:, rs],
                                         y1[:, :])

            # ---- phase E: scores -> exp -> u, s ----
            NG = NRC * NJ

            def emit_scores(g):
                rc, j = divmod(g, NJ)
                rs = slice(rc * RC, (rc + 1) * RC)
                ps = psA[g % 2].tile([128, 2, RC], F32, tag="ps",
                                     name=f"sc{g}")
                for bidx in range(2):
                    for dc in range(NDC):
                        nc.tensor.matmul(
                            ps[:, bidx, :],
                            embtn_sb[:, j, bidx, dc, :],
                            bnnT[dc][:, rs],
                            start=(dc == 0), stop=(dc == NDC - 1),
                        )
                return ps

            acc2 = None
            psu = None
            pend = None
            ps_cur = emit_scores(0)
            for g in range(NG):
                rc, j = divmod(g, NJ)
                if j == 0:
                    if pend is not None:
                        # previous chunk's u copies, on DVE so the scalar
                        # exp stream is not delayed at the boundary
                        psu_old, rc_old = pend
                        for rsub in range(4):
                            ur = outp.tile([128, D], F32, tag="ur")
                            nc.vector.tensor_copy(ur[:, :], psu_old[rsub][:, :])
                            r0 = (rc_old * 4 + rsub) * 128
                            nc.sync.dma_start(out=u_d[r0:r0 + 128, :],
                                              in_=ur[:, :])
                        pend = None
                    acc2 = accp.tile([128, 2, RC], F32, tag="acc2",
                                     name=f"acc2_{rc}")
                    nc.vector.memset(acc2, 0.0)
                    if rc + 1 < NRC:
                        emit_norm_a(rc + 1)
                    psu = [psB.tile([128, D], F32, tag="psB",
                                    name=f"psu{rc}_{i}") for i in range(4)]
                if j == 4 and rc + 1 < NRC:
                    emit_norm_b(rc + 1)
                ps_nxt = emit_scores(g + 1) if g + 1 < NG else None
                expt2 = expp.tile([128, 2, RC], F8, tag="expt")
                for bidx in range(2):
                    nc.scalar.activation(
                        out=expt2[:, bidx, :], in_=ps_cur[:, bidx, :],
                        func=mybir.ActivationFunctionType.Exp,
                        scale=1.0 / (VQ_TEMP * NSCALE),
                    )
                for rsub in range(4):
                    nc.tensor.matmul(
                        psu[rsub][:, :],
                        expt2[:, :, rsub * 128:(rsub + 1) * 128],
                        emb_sb[:, j, :, :],
                        perf_mode=mybir.MatmulPerfMode.DoubleRow,
                        start=(j == 0), stop=(j == NJ - 1),
                    )
                nc.vector.tensor_add(acc2[:, :, :], acc2[:, :, :],
                                     expt2[:, :, :])
                ps_cur = ps_nxt
                if j == NJ - 1:
                    # epilogue for this row chunk. The final chunk copies u
                    # on the (now idle) scalar engine immediately; earlier
                    # chunks defer their copies to the next chunk's start.
                    if rc == NRC - 1:
                        for rsub in range(4):
                            ur = outp.tile([128, D], F32, tag="ur")
                            if rsub < 2:
                                nc.vector.tensor_copy(ur[:, :], psu[rsub][:, :])
                            else:
                                nc.scalar.copy(ur[:, :], psu[rsub][:, :])
                            r0 = (rc * 4 + rsub) * 128
                            nc.sync.dma_start(out=u_d[r0:r0 + 128, :],
                                              in_=ur[:, :])
                    else:
                        pend = (psu, rc)
                    sacc = sqp.tile([128, RC], BF16, tag="sacc",
                                    name=f"sacc{rc}")
                    nc.vector.tensor_add(sacc[:, :], acc2[:, 0, :],
                                         acc2[:, 1, :])
                    sps = psA[g % 2].tile([128, 2, RC], F32, tag="ps",
                                          name=f"sps{rc}")
                    nc.tensor.matmul(sps[0:1, 0, :], ones_bf[:, 0:1],
                                     sacc[:, :], start=True, stop=True)
                    srow = nrp.tile([1, RC], F32, tag="srow", name=f"srow{rc}")
                    nc.vector.tensor_copy(srow[:, :], sps[0:1, 0, :])
                    nc.sync.dma_start(out=s_d[rc:rc + 1, :], in_=srow[0:1, :])

    nc.compile()
    _split_sync_waits(nc)
    return nc


_NC = None


def kernel(audio_kw, W_proj, b_proj, bn_gamma, bn_beta, emb):
    global _NC
    audio_kw = np.asarray(audio_kw, dtype=np.float32)
    W_proj = np.asarray(W_proj, dtype=np.float32)
    bn_gamma = np.asarray(bn_gamma, dtype=np.float32)
    bn_beta = np.asarray(bn_beta, dtype=np.float32)
    emb = np.asarray(emb, dtype=np.float32)

    # host prep: partition-major device layouts
    audioT = np.ascontiguousarray(
        audio_kw.reshape(NRC, RC, NKC, 128).transpose(3, 0, 2, 1)
    ).astype(ml_dtypes.bfloat16)
    wb = np.ascontiguousarray(
        W_proj.reshape(NKC, 128, D).transpose(1, 0, 2)
    ).astype(ml_dtypes.bfloat16)
    # prescale gamma/beta so bn row norms^2 land near 1.0 (the device
    # computes rsqrt(norm^2) by seed-1 Newton; cos is scale-invariant)
    rho = 1.0 / np.sqrt(np.sum(bn_gamma ** 2 + bn_beta ** 2))
    gammab = np.ascontiguousarray((bn_gamma * rho).reshape(NDC, 128).T)
    betab = np.ascontiguousarray((bn_beta * rho).reshape(NDC, 128).T)
    # Newton seed for rstd: population variance of proj_d is ~|W_:,d|^2
    seedb = np.ascontiguousarray(
        (1.0 / np.sqrt(np.sum(W_proj ** 2, axis=0) + BN_EPS))
        .reshape(NDC, 128).T).astype(np.float32)

    norms = np.linalg.norm(emb, axis=1, keepdims=True)
    emb_n = emb / norms
    vtot = VS * NCORES
    embTn_pad = np.zeros((D, vtot), dtype=np.float32)
    embTn_pad[:, :V] = emb_n.T * NSCALE
    emb_pad = np.zeros((vtot, D), dtype=np.float32)
    emb_pad[:V] = emb * ESCALE

    in_maps = []
    for c in range(NCORES):
        # [dc, p, j, b, q] -> [p, j, b, dc, q]
        etn = np.ascontiguousarray(
            embTn_pad[:, c * VS:(c + 1) * VS]
            .reshape(NDC, 128, NJ, 2, 128).transpose(1, 2, 3, 0, 4)
        ).astype(ml_dtypes.float8_e4m3)
        # [j, b, p, d] -> [p, j, b, d]
        eb = np.ascontiguousarray(
            emb_pad[c * VS:(c + 1) * VS]
            .reshape(NJ, 2, 128, D).transpose(2, 0, 1, 3)
        ).astype(ml_dtypes.float8_e4m3)
        in_maps.append({
            "audioTb": audioT, "wb": wb, "gammab": gammab, "betab": betab,
            "rstdseed": seedb, "embTnb": etn, "embb": eb,
        })

    if _NC is None:
        _NC = _build_kernel()
    _patch_upload_artifacts()
    res = run_bass_kernel_spmd(_NC, in_maps, core_ids=list(range(NCORES)))

    u_tot = np.zeros((R, D), dtype=np.float64)
    s_tot = np.zeros((R,), dtype=np.float64)
    for c in range(NCORES):
        u_tot += res.results[c]["u"].astype(np.float64)
        s_tot += res.results[c]["s"].reshape(R).astype(np.float64)
    s_tot -= NPAD  # zero pad rows contribute exactly exp(0)=1 each
    out = (u_tot / ESCALE / s_tot[:, None]).astype(np.float32)
    return out.reshape(B, K, D)


# revision 22
# speedup vs baseline: 1.0600x; 1.0174x over previous
"""CascadedBranch (retrieval_knn) Trainium2 kernel.

Reference computation (B=256, K=8, Da=768, Dt=512, V=49408):
    proj = audio_kw @ W_proj + b_proj          # [B,K,Dt]
    bn   = batchnorm over (B,K) with gamma/beta
    cos  = normalize(bn) @ normalize(emb).T    # [B,K,V]
    prob = softmax(cos / 0.1)
    out  = prob @ emb                          # [B,K,Dt]

Strategy: shard the vocab axis V across the 8 cores (6400 rows each after
padding 49408 -> 51200). Each core computes, for all 2048 rows:
    projT (W.T @ audio.T, PSUM f32), batchnorm stats via bn_stats/bn_aggr,
    normalized bnT (in [d, row] layout so BN params are per-partition),
    scores sT[v,row] = embT_q8 @ bnT_n (fp8 stationary x bf16 moving),
    expt = exp(scores * 10/64) in fp8e4,
    u = sum_v expt*emb_q8 via fp8 DoubleRow matmuls (2 MACs/PE/cycle),
    s = sum_v expt via DVE accumulate + a 1-column PE matmul reduce.
No max-subtraction is needed: |cos|<=~1 so logits are in [-10.2, 10.2].
Host combines: out = (sum_c u_c / ESCALE) / (sum_c s_c - NPAD)  (the pad
rows contribute exactly exp(0)=1 to s and 0 to u).
b_proj is ignored: a linear bias cancels exactly inside batchnorm.

Row-norm trick: gamma/beta are host-prescaled by 1/sqrt(sum(g^2+b^2)) so
every bn row norm^2 lands near 1.0; the device then computes
rsqrt(norm^2) as two seed-1 Newton steps on the DVE -- no scalar-engine
Sqrt in the steady loop, so the scalar activation table never leaves Exp
and the in-order scalar queue can't head-of-line block on the norm chain.

Schedule: the vocab loop is software-pipelined one pair ahead (including
across row-chunk boundaries, alternating score-psum pools on a global
pair counter) so the scalar exp always hides under the next pair's score
stream. The next row chunk's norm chain runs on DVE/gpsimd during the
current chunk's vocab loop (reduce at pair 0, rsqrt+muls at pair 4).

Everything needed is hardcoded; no sibling imports.
"""

import numpy as np
import ml_dtypes

import concourse.bass as bass
import concourse.bacc as bacc
import concourse.tile as tile
from concourse import mybir
from concourse import bass_isa
from concourse.bass_utils import run_bass_kernel_spmd

F32 = mybir.dt.float32
BF16 = mybir.dt.bfloat16
F8 = mybir.dt.float8e4

B, K, DA, D, V = 256, 8, 768, 512, 49408
R = B * K              # 2048 rows
NCORES = 8
VS = 6400              # per-core vocab shard (padded)
NJ = VS // 256         # 25 vocab pairs of 2x128
NRC = 4                # row chunks of 512
RC = 512
NDC = D // 128         # 4 d-chunks
NKC = DA // 128        # 6 k-chunks
NPAD = VS * NCORES - V  # 1792 zero pad rows (all in core 7's shard)
VQ_TEMP = 0.1
BN_EPS = 1e-5
ESCALE = 256.0         # emb pre-scale for fp8 quantization
NSCALE = 64.0          # emb_n pre-scale for fp8 quantization


def _split_sync_waits(nc):
    """The walrus in this image rejects >1 sem-wait per instruction
    ("Too many sync wait commands"). Legalize by inserting single-wait
    Drain carriers immediately before any multi-wait instruction (same
    engine, same basic block position => identical synchronization)."""
    import orjson
    js = orjson.loads(mybir.module_to_json_bytes(nc.m))
    ctr = 0
    for func in js["functions"]:
        for bb in func["blocks"]:
            out = []
            changed = False
            for inst in bb["instructions"]:
                si = inst.get("sync_info")
                waits = (si or {}).get("on_wait") or []
                if len(waits) > 1:
                    changed = True
                    for w in waits[:-1]:
                        ctr += 1
                        carrier = {
                            "name": f"I-lsw-{ctr}",
                            "opcode": "Drain",
                            "engine": inst["engine"],
                            "ins": [],
                            "outs": [],
                            "sync_info": {"on_wait": [w], "on_update": []},
                        }
                        if "debug" in inst:
                            carrier["debug"] = inst["debug"]
                        out.append(carrier)
                    si["on_wait"] = [waits[-1]]
                out.append(inst)
            if changed:
                bb["instructions"] = out
    nc.m = mybir.module_from_json_bytes(orjson.dumps(js))
    return nc


def _patch_upload_artifacts():
    import concourse.bass_utils as bu
    bu.upload_artifacts = lambda tmpdir: "local://" + str(tmpdir)


def _build_kernel():
    nc = bacc.Bacc("TRN2", target_bir_lowering=False)

    # inputs, host-prepped into [128, ...] partition-major layouts
    audio_d = nc.dram_tensor("audioTb", [128, NRC, NKC, RC], BF16,
                             kind="ExternalInput")
    w_d = nc.dram_tensor("wb", [128, NKC, D], BF16, kind="ExternalInput")
    gamma_d = nc.dram_tensor("gammab", [128, NDC], F32, kind="ExternalInput")
    seed_d = nc.dram_tensor("rstdseed", [128, NDC], F32, kind="ExternalInput")
    beta_d = nc.dram_tensor("betab", [128, NDC], F32, kind="ExternalInput")
    # embTnb[p, j, b, dc, q] = NSCALE*normalize(emb)[vshard, d] (fp8e4)
    embtn_d = nc.dram_tensor("embTnb", [128, NJ, 2, NDC, 128], F8,
                             kind="ExternalInput")
    # embb[p, j, b, d] = ESCALE*emb[shard + j*256+b*128+p, d] in fp8e4
    emb_d = nc.dram_tensor("embb", [128, NJ, 2, D], F8, kind="ExternalInput")
    u_d = nc.dram_tensor("u", [R, D], F32, kind="ExternalOutput")
    s_d = nc.dram_tensor("s", [NRC, RC], F32, kind="ExternalOutput")

    with tile.TileContext(nc) as tc:
        with (
            tc.tile_pool(name="consts", bufs=1) as consts,
            tc.tile_pool(name="persist", bufs=1) as persist,
            tc.tile_pool(name="sqp", bufs=4) as sqp,
            tc.tile_pool(name="nrp", bufs=2) as nrp,
            tc.tile_pool(name="expp", bufs=12) as expp,
            tc.tile_pool(name="accp", bufs=2) as accp,
            tc.tile_pool(name="outp", bufs=4) as outp,
            tc.tile_pool(name="psA0", bufs=1, space="PSUM") as psA0,
            tc.tile_pool(name="psA1", bufs=1, space="PSUM") as psA1,
            tc.tile_pool(name="psB", bufs=4, space="PSUM") as psB,
        ):
            psA = (psA0, psA1)
            # ---- load inputs: few big DMAs for max HBM rate ----
            w_sb = consts.tile([128, NKC, D], BF16, tag="w")
            nc.sync.dma_start(out=w_sb[:, :, :], in_=w_d[:, :, :])
            audio_sb = consts.tile([128, NRC, NKC, RC], BF16, tag="audio")
            for rc in range(NRC):
                for h in range(2):
                    ks = slice(h * 3, h * 3 + 3)
                    nc.sync.dma_start(out=audio_sb[:, rc, ks, :],
                                      in_=audio_d[:, rc, ks, :])
            gamma_sb = consts.tile([128, NDC], F32, tag="gamma")
            nc.sync.dma_start(out=gamma_sb[:, :], in_=gamma_d[:, :])
            beta_sb = consts.tile([128, NDC], F32, tag="beta")
            nc.sync.dma_start(out=beta_sb[:, :], in_=beta_d[:, :])
            seed_sb = consts.tile([128, NDC], F32, tag="seed")
            nc.sync.dma_start(out=seed_sb[:, :], in_=seed_d[:, :])
            embtn_sb = consts.tile([128, NJ, 2, NDC, 128], F8, tag="embtn")
            emb_sb = consts.tile([128, NJ, 2, D], F8, tag="emb")
            for ch in range(2):
                jj = slice(ch * 12, 25 if ch else 12)
                nc.sync.dma_start(out=embtn_sb[:, jj, :, :, :],
                                  in_=embtn_d[:, jj, :, :, :])
                nc.sync.dma_start(out=emb_sb[:, jj, :, :], in_=emb_d[:, jj, :, :])

            ones_bf = consts.tile([128, 1], BF16, tag="ones_bf")
            nc.vector.memset(ones_bf, 1.0)
            ones_row = consts.tile([1, 128], BF16, tag="ones_row")
            nc.vector.memset(ones_row, 1.0)

            projT = [persist.tile([128, R], BF16, tag=f"projT{dc}",
                                  name=f"projT{dc}") for dc in range(NDC)]
            stats = [persist.tile([128, NRC, 6], F32, tag=f"stats{dc}",
                                  name=f"stats{dc}") for dc in range(NDC)]
            bnnT = [persist.tile([128, R], BF16, tag=f"bnnT{dc}",
                                 name=f"bnnT{dc}") for dc in range(NDC)]
            sdc, bdc = [None] * NDC, [None] * NDC
            sq0 = [None] * NDC
            rs0 = slice(0, RC)

            # ---- phase B: projT = W.T @ audio.T (rc-major so each audio
            # DMA chunk feeds two back-to-back groups). bn_stats on DVE,
            # psum->SBUF copy on the scalar engine.
            for rc in range(NRC):
                rs = slice(rc * RC, (rc + 1) * RC)
                for dch in range(2):
                    g = rc * 2 + dch
                    ps = psA[g % 2].tile([128, 2, RC], F32, tag="ps")
                    for b in range(2):
                        dc = dch * 2 + b
                        for a in range(NKC):
                            nc.tensor.matmul(
                                ps[:, b, :],
                                w_sb[:, a, dc * 128:(dc + 1) * 128],
                                audio_sb[:, rc, a, :],
                                start=(a == 0),
                                stop=(a == NKC - 1),
                            )
                    for b in range(2):
                        dc = dch * 2 + b
                        nc.vector.bn_stats(out=stats[dc][:, rc, :], in_=ps[:, b, :])
                        nc.scalar.copy(projT[dc][:, rs], ps[:, b, :])

            # ---- phase C: BN affine params, all d-chunks batched, DVE
            # only. rstd = rsqrt(var) via 2 Newton steps from the host
            # seed 1/sqrt(sum_k W_kd^2) (~3% off, exact after 2 steps).
            mv4 = persist.tile([128, NDC, 2], F32, tag="mv4")
            for dc in range(NDC):
                nc.vector.bn_aggr(out=mv4[:, dc, :], in_=stats[dc][:, :, :])
            var4 = mv4[:, :, 1]
            mean4 = mv4[:, :, 0]
            y4 = persist.tile([128, NDC], F32, tag="y4")
            t4 = persist.tile([128, NDC], F32, tag="t4")
            nc.vector.tensor_copy(y4[:, :], seed_sb[:, :])
            for _ in range(2):
                nc.vector.tensor_mul(t4[:, :], var4, y4[:, :])
                nc.vector.tensor_mul(t4[:, :], t4[:, :], y4[:, :])
                nc.vector.tensor_scalar(
                    out=t4[:, :], in0=t4[:, :], scalar1=-0.5, scalar2=1.5,
                    op0=mybir.AluOpType.mult, op1=mybir.AluOpType.add,
                )
                nc.vector.tensor_mul(y4[:, :], y4[:, :], t4[:, :])
            s_aff4 = persist.tile([128, NDC], F32, tag="saff4")
            nc.vector.tensor_mul(s_aff4[:, :], y4[:, :], gamma_sb[:, :])
            b_aff4 = persist.tile([128, NDC], F32, tag="baff4")
            nc.vector.tensor_mul(b_aff4[:, :], mean4, s_aff4[:, :])
            nc.vector.tensor_tensor(
                out=b_aff4[:, :], in0=beta_sb[:, :], in1=b_aff4[:, :],
                op=mybir.AluOpType.subtract,
            )
            for dc in range(NDC):
                sdc[dc] = s_aff4[:, dc:dc + 1]
                bdc[dc] = b_aff4[:, dc:dc + 1]

            # ---- rc0 norm tail: affine on DVE in parallel with
            # (s*proj+b)^2 on scalar; partition-reduce + broadcast on the
            # (idle) PE; rsqrt via seed-1 Newton on a [1,RC] strip.
            for dc in range(NDC):
                nc.vector.tensor_scalar(
                    out=bnnT[dc][:, rs0], in0=projT[dc][:, rs0],
                    scalar1=sdc[dc], scalar2=bdc[dc],
                    op0=mybir.AluOpType.mult, op1=mybir.AluOpType.add,
                )
                sqt = sqp.tile([128, RC], BF16, tag="sqt", name=f"sq0_{dc}")
                nc.scalar.activation(
                    out=sqt[:, :], in_=projT[dc][:, rs0],
                    func=mybir.ActivationFunctionType.Square,
                    bias=bdc[dc], scale=sdc[dc],
                )
                sq0[dc] = sqt
            n2ps = psA0.tile([128, 2, RC], F32, tag="ps", name="n2ps")
            for dc in range(NDC):
                nc.tensor.matmul(
                    n2ps[0:1, 0, :], ones_bf[:, 0:1], sq0[dc][:, :],
                    start=(dc == 0), stop=(dc == NDC - 1),
                )
            yr = nrp.tile([1, RC], F32, tag="yr")
            tr = nrp.tile([1, RC], F32, tag="tr")
            nc.vector.tensor_scalar(
                out=yr[:, :], in0=n2ps[0:1, 0, :], scalar1=-0.5, scalar2=1.5,
                op0=mybir.AluOpType.mult, op1=mybir.AluOpType.add,
            )
            nc.vector.tensor_mul(tr[:, :], n2ps[0:1, 0, :], yr[:, :])
            nc.vector.tensor_mul(tr[:, :], tr[:, :], yr[:, :])
            nc.vector.tensor_scalar(
                out=tr[:, :], in0=tr[:, :], scalar1=-0.5, scalar2=1.5,
                op0=mybir.AluOpType.mult, op1=mybir.AluOpType.add,
            )
            nc.vector.tensor_mul(yr[:, :], yr[:, :], tr[:, :])
            ybf = nrp.tile([1, RC], BF16, tag="ybf")
            nc.vector.tensor_copy(ybf[:, :], yr[:, :])
            rbcps = psA1.tile([128, 2, RC], F32, tag="ps", name="rbcps")
            nc.tensor.matmul(rbcps[:, 0, :], ones_row[:, :], ybf[:, :],
                             start=True, stop=True)
            for dc in range(NDC):
                nc.vector.tensor_mul(bnnT[dc][:, rs0], bnnT[dc][:, rs0],
                                     rbcps[:, 0, :])

            norm_state = {}

            def emit_norm_a(rc):
                # affine + squares + partition reduce (DVE + gpsimd only)
                rs = slice(rc * RC, (rc + 1) * RC)
                sqa = sqp.tile([128, RC], F32, tag="sqa", name=f"sqa{rc}")
                for dc in range(NDC):
                    nc.vector.tensor_scalar(
                        out=bnnT[dc][:, rs], in0=projT[dc][:, rs],
                        scalar1=sdc[dc], scalar2=bdc[dc],
                        op0=mybir.AluOpType.mult, op1=mybir.AluOpType.add,
                    )
                    sqt = sqp.tile([128, RC], F32, tag="sqf", name=f"sqf{rc}_{dc}")
                    nc.vector.tensor_mul(sqt[:, :], bnnT[dc][:, rs],
                                         bnnT[dc][:, rs])
                    if dc == 0:
                        sq_first = sqt
                    elif dc == 1:
                        nc.vector.tensor_add(sqa[:, :], sq_first[:, :], sqt[:, :])
                    else:
                        nc.vector.tensor_add(sqa[:, :], sqa[:, :], sqt[:, :])
                n2r = nrp.tile([128, RC], F32, tag="n2r", name=f"n2r{rc}")
                nc.gpsimd.partition_all_reduce(
                    n2r[:, :], sqa[:, :], channels=128,
                    reduce_op=bass_isa.ReduceOp.add,
                )
                norm_state[rc] = n2r

            def emit_norm_b(rc):
                # rsqrt(n2) via two seed-1 Newton steps (norms ~1 by the
                # host gamma/beta prescale), then scale bnnT. DVE only.
                rs = slice(rc * RC, (rc + 1) * RC)
                x = norm_state.pop(rc)
                y1 = nrp.tile([128, RC], F32, tag="rbc", name=f"y1_{rc}")
                nc.vector.tensor_scalar(
                    out=y1[:, :], in0=x[:, :], scalar1=-0.5, scalar2=1.5,
                    op0=mybir.AluOpType.mult, op1=mybir.AluOpType.add,
                )
                t = nrp.tile([128, RC], F32, tag="nt", name=f"nt{rc}")
                nc.vector.tensor_mul(t[:, :], x[:, :], y1[:, :])
                nc.vector.tensor_mul(t[:, :], t[:, :], y1[:, :])
                nc.vector.tensor_scalar(
                    out=t[:, :], in0=t[:, :], scalar1=-0.5, scalar2=1.5,
                    op0=mybir.AluOpType.mult, op1=mybir.AluOpType.add,
                )
                nc.vector.tensor_mul(y1[:, :], y1[:, :], t[:, :])
                for dc in range(NDC):
                    nc.vector.tensor_mul(bnnT[dc][:, rs], bnnT[dc][:, rs],
                                         y1[:, :])

            # ---- phase E: scores -> exp -> u, s ----
            NG = NRC * NJ

            def emit_scores(g):
                rc, j = divmod(g, NJ)
                rs = slice(rc * RC, (rc + 1) * RC)
                ps = psA[g % 2].tile([128, 2, RC], F32, tag="ps",
                                     name=f"sc{g}")
                for bidx in range(2):
                    for dc in range(NDC):
                        nc.tensor.matmul(
                            ps[:, bidx, :],
                            embtn_sb[:, j, bidx, dc, :],
                            bnnT[dc][:, rs],
                            start=(dc == 0), stop=(dc == NDC - 1),
                        )
                return ps

            acc2 = None
            psu = None
            pend = None
            ps_cur = emit_scores(0)
            for g in range(NG):
                rc, j = divmod(g, NJ)
                if j == 0:
                    if pend is not None:
                        # previous chunk's u copies, on DVE so the scalar
                        # exp stream is not delayed at the boundary
                        psu_old, rc_old = pend
                        for rsub in range(4):
                            ur = outp.tile([128, D], F32, tag="ur")
                            nc.vector.tensor_copy(ur[:, :], psu_old[rsub][:, :])
                            r0 = (rc_old * 4 + rsub) * 128
                            nc.sync.dma_start(out=u_d[r0:r0 + 128, :],
                                              in_=ur[:, :])
                        pend = None
                    acc2 = accp.tile([128, 2, RC], F32, tag="acc2",
                                     name=f"acc2_{rc}")
                    nc.vector.memset(acc2, 0.0)
                    if rc + 1 < NRC:
                        emit_norm_a(rc + 1)
                    psu = [psB.tile([128, D], F32, tag="psB",
                                    name=f"psu{rc}_{i}") for i in range(4)]
                if j == 4 and rc + 1 < NRC:
                    emit_norm_b(rc + 1)
                ps_nxt = emit_scores(g + 1) if g + 1 < NG else None
                expt2 = expp.tile([128, 2, RC], F8, tag="expt")
                for bidx in range(2):
                    nc.scalar.activation(
                        out=expt2[:, bidx, :], in_=ps_cur[:, bidx, :],
                        func=mybir.ActivationFunctionType.Exp,
                        scale=1.0 / (VQ_TEMP * NSCALE),
                    )
                for rsub in range(4):
                    nc.tensor.matmul(
                        psu[rsub][:, :],
                        expt2[:, :, rsub * 128:(rsub + 1) * 128],
                        emb_sb[:, j, :, :],
                        perf_mode=mybir.MatmulPerfMode.DoubleRow,
                        start=(j == 0), stop=(j == NJ - 1),
                    )
                if j < NJ - 1:
                    nc.vector.tensor_add(acc2[:, :, :], acc2[:, :, :],
                                         expt2[:, :, :])
                ps_cur = ps_nxt
                if j == NJ - 2:
                    # partial softmax-denominator over pairs 0..23 (pair 24
                    # is folded into the PE s-matmul below)
                    sacc = sqp.tile([128, RC], BF16, tag="sacc",
                                    name=f"sacc{rc}")
                    nc.vector.tensor_add(sacc[:, :], acc2[:, 0, :],
                                         acc2[:, 1, :])
                    sacc_pre = sacc
                if j == NJ - 1:
                    # epilogue for this row chunk. The final chunk copies u
                    # on the (now idle) scalar engine immediately; earlier
                    # chunks defer their copies to the next chunk's start.
                    if rc == NRC - 1:
                        for rsub in range(4):
                            ur = outp.tile([128, D], F32, tag="ur")
                            if rsub < 2:
                                nc.vector.tensor_copy(ur[:, :], psu[rsub][:, :])
                            else:
                                nc.scalar.copy(ur[:, :], psu[rsub][:, :])
                            r0 = (rc * 4 + rsub) * 128
                            nc.sync.dma_start(out=u_d[r0:r0 + 128, :],
                                              in_=ur[:, :])
                    else:
                        pend = (psu, rc)
                    sps = psA[g % 2].tile([128, 2, RC], F32, tag="ps",
                                          name=f"sps{rc}")
                    nc.tensor.matmul(sps[0:1, 0, :], ones_bf[:, 0:1],
                                     sacc_pre[:, :], start=True, stop=False)
                    for bidx in range(2):
                        nc.tensor.matmul(sps[0:1, 0, :], ones_bf[:, 0:1],
                                         expt2[:, bidx, :], start=False,
                                         stop=(bidx == 1))
                    srow = nrp.tile([1, RC], F32, tag="srow", name=f"srow{rc}")
                    nc.vector.tensor_copy(srow[:, :], sps[0:1, 0, :])
                    nc.sync.dma_start(out=s_d[rc:rc + 1, :], in_=srow[0:1, :])

    nc.compile()
    _split_sync_waits(nc)
    return nc


_NC = None


def kernel(audio_kw, W_proj, b_proj, bn_gamma, bn_beta, emb):
    global _NC
    audio_kw = np.asarray(audio_kw, dtype=np.float32)
    W_proj = np.asarray(W_proj, dtype=np.float32)
    bn_gamma = np.asarray(bn_gamma, dtype=np.float32)
    bn_beta = np.asarray(bn_beta, dtype=np.float32)
    emb = np.asarray(emb, dtype=np.float32)

    # host prep: partition-major device layouts
    audioT = np.ascontiguousarray(
        audio_kw.reshape(NRC, RC, NKC, 128).transpose(3, 0, 2, 1)
    ).astype(ml_dtypes.bfloat16)
    wb = np.ascontiguousarray(
        W_proj.reshape(NKC, 128, D).transpose(1, 0, 2)
    ).astype(ml_dtypes.bfloat16)
    # prescale gamma/beta so bn row norms^2 land near 1.0 (the device
    # computes rsqrt(norm^2) by seed-1 Newton; cos is scale-invariant)
    rho = 1.0 / np.sqrt(np.sum(bn_gamma ** 2 + bn_beta ** 2))
    gammab = np.ascontiguousarray((bn_gamma * rho).reshape(NDC, 128).T)
    betab = np.ascontiguousarray((bn_beta * rho).reshape(NDC, 128).T)
    # Newton seed for rstd: population variance of proj_d is ~|W_:,d|^2
    seedb = np.ascontiguousarray(
        (1.0 / np.sqrt(np.sum(W_proj ** 2, axis=0) + BN_EPS))
        .reshape(NDC, 128).T).astype(np.float32)

    norms = np.linalg.norm(emb, axis=1, keepdims=True)
    emb_n = emb / norms
    vtot = VS * NCORES
    embTn_pad = np.zeros((D, vtot), dtype=np.float32)
    embTn_pad[:, :V] = emb_n.T * NSCALE
    emb_pad = np.zeros((vtot, D), dtype=np.float32)
    emb_pad[:V] = emb * ESCALE

    in_maps = []
    for c in range(NCORES):
        # [dc, p, j, b, q] -> [p, j, b, dc, q]
        etn = np.ascontiguousarray(
            embTn_pad[:, c * VS:(c + 1) * VS]
            .reshape(NDC, 128, NJ, 2, 128).transpose(1, 2, 3, 0, 4)
        ).astype(ml_dtypes.float8_e4m3)
        # [j, b, p, d] -> [p, j, b, d]
        eb = np.ascontiguousarray(
            emb_pad[c * VS:(c + 1) * VS]
            .reshape(NJ, 2, 128, D).transpose(2, 0, 1, 3)
        ).astype(ml_dtypes.float8_e4m3)
        in_maps.append({
            "audioTb": audioT, "wb": wb, "gammab": gammab, "betab": betab,
            "rstdseed": seedb, "embTnb": etn, "embb": eb,
        })

    if _NC is None:
        _NC = _build_kernel()
    _patch_upload_artifacts()
    res = run_bass_kernel_spmd(_NC, in_maps, core_ids=list(range(NCORES)))

    u_tot = np.zeros((R, D), dtype=np.float64)
    s_tot = np.zeros((R,), dtype=np.float64)
    for c in range(NCORES):
        u_tot += res.results[c]["u"].astype(np.float64)
        s_tot += res.results[c]["s"].reshape(R).astype(np.float64)
    s_tot -= NPAD  # zero pad rows contribute exactly exp(0)=1 each
    out = (u_tot / ESCALE / s_tot[:, None]).astype(np.float32)
    return out.reshape(B, K, D)
